# revision 1
# baseline (speedup 1.0000x reference)
"""Channel-attention Trainium2 kernel (Bass/Tile, 8 NeuronCores).

The reference computes, after un-permuting the V path:

    out[b,c,t,f] = sum_k w[b, f//64, c, k] * x[b,k,t,f]
    w[b,h]       = softmax_k( (q_h q-rows) @ (k_h rows)^T / 8 )
    q            = mean_t(x[b]) @ Wq.T + bq,   k = mean_t(x[b]) @ Wk.T

i.e. a per-(batch, head) 128x128 channel-mixing matmul over the full
(T x 64) feature block, fed by a tiny pooled q/k path.

Sharding: 8 cores = (batch b in {0,1}) x (T-quarter q in {0..3}); each core
owns x[b, :, q*128:(q+1)*128, :] (32 MB, fully contiguous per-channel rows).

Phase 1 (device): per-core partial sum over its t-slice -> (128, 512),
loads alternating the two HWDGE rings (~300 GB/s/core read).
Host: combine 4 partials per batch into xm[b] (tiny, 256 KB).
Phase 2 (device): per-core replicated pooled path (q/k projections, per-head
softmax, transpose), then the streaming per-head channel-mix matmuls:
2 MB input tiles on the SP ring, 2 MB output tiles on the ACT ring
(~358 GB/s/core aggregate = HBM-per-NC cap), one N=512 matmul per head
into a dedicated PSUM bank.

Measured (repeat-delta, 8 cores): phase1 ~100-115 us, phase2 ~175-210 us,
total ~290-325 us vs a ~286 us memory-roofline floor (96 MB/core of
irreducible HBM traffic: x read twice + out written once).
"""

import numpy as np

import concourse.bacc as bacc
import concourse.mybir as mybir
import concourse.tile as tile
from concourse.bass import ds, ts
from concourse.bass_utils import run_bass_kernel_spmd
from concourse.masks import make_identity

B, C, T, F = 2, 128, 512, 512
H = 8
D = F // H            # 64 features per head
NCORES = 8
TQ = T // 4           # 128 t's per core
FCHUNKS = F // 128    # 4
F32 = mybir.dt.float32

# test.py can flip this to get NTFF profiling / exec_time_ns
TRACE = False
LAST_PROFILE = {}

_CACHE = {}


def _make_nc():
    return bacc.Bacc(
        "TRN2", target_bir_lowering=False, debug=False, num_devices=NCORES
    )


def _build_phase1(repeat=1):
    """Partial sum over the core's 128 t's: ps[c, f] = sum_t xs[c, t, f].

    repeat>1 re-runs the streaming pass (same reads) for benchmarking only.
    """
    nc = _make_nc()
    xs = nc.dram_tensor("xs", [C, TQ, F], F32, kind="ExternalInput")
    ps = nc.dram_tensor("ps", [C, F], F32, kind="ExternalOutput")
    TT = 8
    with tile.TileContext(nc) as tc:
        with (
            tc.tile_pool(name="xin", bufs=3) as xpool,
            tc.tile_pool(name="tmp", bufs=2) as tpool,
            tc.tile_pool(name="accp", bufs=1) as apool,
        ):
            acc = apool.tile([C, F], F32, name="acc")
            for rep in range(repeat):
                for it in range(TQ // TT):
                    xt = xpool.tile([C, TT, F], F32, name="xt")
                    # alternate the two HWDGE rings (SP / ACT) for read BW
                    eng = nc.sync if it % 2 == 0 else nc.scalar
                    eng.dma_start(xt[:], xs[:, ts(it, TT), :])
                    view = xt[:].rearrange("c t f -> c f t")
                    if rep == 0 and it == 0:
                        nc.vector.reduce_sum(acc[:], view, axis=mybir.AxisListType.X)
                    else:
                        red = tpool.tile([C, F], F32, name="red")
                        nc.vector.reduce_sum(red[:], view, axis=mybir.AxisListType.X)
                        nc.vector.tensor_add(acc[:], acc[:], red[:])
            nc.sync.dma_start(ps[:], acc[:])
    nc.finalize()
    return nc


def _build_phase2(repeat=1):
    """Pooled q/k path (replicated per core) + streaming per-head matmuls.

    repeat>1 re-runs the streaming pass (same reads/writes), bench only.
    """
    nc = _make_nc()
    xs = nc.dram_tensor("xs", [C, TQ, F], F32, kind="ExternalInput")     # (k,t,f)
    xmt = nc.dram_tensor("xmt", [F, C], F32, kind="ExternalInput")       # xm[b].T
    wqt = nc.dram_tensor("wqt", [F, F], F32, kind="ExternalInput")       # Wq.T (f,j)
    wkt = nc.dram_tensor("wkt", [F, F], F32, kind="ExternalInput")       # Wk.T (f,j)
    bqr = nc.dram_tensor("bqr", [C, FCHUNKS], F32, kind="ExternalInput")  # bq (4,128).T
    out = nc.dram_tensor("out", [C, TQ, F], F32, kind="ExternalOutput")  # (c,t,f)
    TT = 8  # t's per DMA tile (2 MB transfers)
    with tile.TileContext(nc) as tc:
        with (
            tc.tile_pool(name="const", bufs=1) as const,
            tc.tile_pool(name="wts", bufs=1) as wts,
            tc.tile_pool(name="small", bufs=2) as small,
            tc.tile_pool(name="xin", bufs=3) as xpool,
            tc.tile_pool(name="oout", bufs=3) as opool,
            tc.tile_pool(name="pqk", bufs=2, space="PSUM") as pqk,
            tc.tile_pool(name="pbig", bufs=6, space="PSUM") as pbig,
        ):
            ident = const.tile([128, 128], F32, name="ident")
            make_identity(nc, ident)
            wqt_sb = const.tile([128, FCHUNKS, F], F32, name="wqt_sb")
            nc.sync.dma_start(wqt_sb[:], wqt.rearrange("(o p) j -> p o j", p=128))
            wkt_sb = const.tile([128, FCHUNKS, F], F32, name="wkt_sb")
            nc.sync.dma_start(wkt_sb[:], wkt.rearrange("(o p) j -> p o j", p=128))
            xmt_sb = const.tile([128, FCHUNKS, C], F32, name="xmt_sb")
            nc.sync.dma_start(xmt_sb[:], xmt.rearrange("(o p) c -> p o c", p=128))
            bq_sb = const.tile([C, FCHUNKS], F32, name="bq_sb")
            nc.sync.dma_start(bq_sb[:], bqr[:])

            # qT[j, c] = sum_f Wq[j, f] xm[c, f] + bq[j]; kT likewise (no bias).
            qt_sb = wts.tile([128, FCHUNKS, C], F32, name="qt_sb")
            kt_sb = wts.tile([128, FCHUNKS, C], F32, name="kt_sb")
            for jc in range(FCHUNKS):
                psq = pqk.tile([128, C], F32, name="psq", tag="smallps")
                for fc in range(FCHUNKS):
                    nc.tensor.matmul(
                        psq[:],
                        wqt_sb[:, fc, ts(jc, 128)],
                        xmt_sb[:, fc, :],
                        start=(fc == 0),
                        stop=(fc == FCHUNKS - 1),
                    )
                nc.scalar.activation(
                    qt_sb[:, jc, :],
                    psq[:],
                    mybir.ActivationFunctionType.Identity,
                    bias=bq_sb[:, jc : jc + 1],
                    scale=1.0,
                )
                psk = pqk.tile([128, C], F32, name="psk", tag="smallps")
                for fc in range(FCHUNKS):
                    nc.tensor.matmul(
                        psk[:],
                        wkt_sb[:, fc, ts(jc, 128)],
                        xmt_sb[:, fc, :],
                        start=(fc == 0),
                        stop=(fc == FCHUNKS - 1),
                    )
                nc.scalar.copy(kt_sb[:, jc, :], psk[:])

            # Per-head attention weights, stored transposed: wT[k2, h, c].
            wt_sb = wts.tile([128, H, C], F32, name="wt_sb")
            for h in range(H):
                jc, off = h // 2, D * (h % 2)
                psa = pqk.tile([C, C], F32, name="psa", tag="smallps")
                nc.tensor.matmul(
                    psa[:],
                    qt_sb[off : off + D, jc, :],
                    kt_sb[off : off + D, jc, :],
                    start=True,
                    stop=True,
                )
                qk = small.tile([C, C], F32, name="qk")
                nc.scalar.mul(qk[:], psa[:], 0.125)  # (d ** -0.25) ** 2 folded
                nmax = small.tile([C, 1], F32, name="nmax")
                nc.vector.reduce_max(
                    nmax[:], qk[:], axis=mybir.AxisListType.X, negate=True
                )
                ex = small.tile([C, C], F32, name="ex")
                nc.scalar.activation(
                    ex[:],
                    qk[:],
                    mybir.ActivationFunctionType.Exp,
                    bias=nmax[:],
                    scale=1.0,
                )
                esum = small.tile([C, 1], F32, name="esum")
                nc.vector.reduce_sum(esum[:], ex[:], axis=mybir.AxisListType.X)
                rsum = small.tile([C, 1], F32, name="rsum")
                nc.vector.reciprocal(rsum[:], esum[:])
                wsm = small.tile([C, C], F32, name="wsm")
                nc.vector.tensor_scalar_mul(wsm[:], ex[:], rsum[:])
                pst = pqk.tile([C, C], F32, name="pst", tag="smallps")
                nc.tensor.transpose(pst[:], wsm[:], ident[:])
                nc.vector.tensor_copy(wt_sb[:, h, :], pst[:])

            # Streaming channel-mix. Inputs ride the SP HWDGE ring, outputs
            # the ACT ring, so both directions stream concurrently. Per 2MB
            # tile: one N=512 matmul per head into a per-head PSUM bank
            # (out[c, (d, t)] via the (d, t) access pattern on xt), then a
            # per-head interleaving copy into the (t, f) staging tile.
            for rep in range(repeat):
                for it in range(TQ // TT):
                    xt = xpool.tile([C, TT, F], F32, name="xt")
                    nc.sync.dma_start(xt[:], xs[:, ts(it, TT), :])
                    ot = opool.tile([C, TT, F], F32, name="ot")
                    for h in range(H):
                        pso = pbig.tile([C, D, TT], F32, name="pso")
                        nc.tensor.matmul(
                            pso[:],
                            wt_sb[:, h, :],
                            xt[:, :, ds(D * h, D)].rearrange("k t d -> k d t"),
                            start=True,
                            stop=True,
                        )
                        nc.vector.tensor_copy(
                            ot[:, :, ds(D * h, D)],
                            pso[:].rearrange("c d t -> c t d"),
                        )
                    nc.scalar.dma_start(out[:, ts(it, TT), :], ot[:])
    nc.finalize()
    return nc


def _programs():
    if "p1" not in _CACHE:
        _CACHE["p1"] = _build_phase1()
        _CACHE["p2"] = _build_phase2()
    return _CACHE["p1"], _CACHE["p2"]


def kernel(x, Wq, bq, Wk):
    x = np.ascontiguousarray(np.asarray(x), dtype=np.float32)
    Wq = np.asarray(Wq, dtype=np.float32)
    bq = np.asarray(bq, dtype=np.float32)
    Wk = np.asarray(Wk, dtype=np.float32)
    assert x.shape == (B, C, T, F)

    nc1, nc2 = _programs()
    core_ids = list(range(NCORES))

    xs_list = []
    for i in range(NCORES):
        b, q = divmod(i, 4)
        xs_list.append(np.ascontiguousarray(x[b, :, q * TQ : (q + 1) * TQ, :]))

    r1 = run_bass_kernel_spmd(
        nc1, [{"xs": xs_list[i]} for i in range(NCORES)], core_ids, trace=TRACE
    )
    LAST_PROFILE["phase1_ns"] = r1.exec_time_ns

    xm = np.zeros((B, C, F), np.float64)
    for i in range(NCORES):
        xm[i // 4] += r1.results[i]["ps"].astype(np.float64)
    xm = (xm / T).astype(np.float32)

    xmT = [np.ascontiguousarray(xm[b].T) for b in range(B)]
    WqT = np.ascontiguousarray(Wq.T)
    WkT = np.ascontiguousarray(Wk.T)
    bqr = np.ascontiguousarray(bq.reshape(FCHUNKS, 128).T)

    in_maps = []
    for i in range(NCORES):
        b = i // 4
        in_maps.append(
            {"xs": xs_list[i], "xmt": xmT[b], "wqt": WqT, "wkt": WkT, "bqr": bqr}
        )
    r2 = run_bass_kernel_spmd(nc2, in_maps, core_ids, trace=TRACE)
    LAST_PROFILE["phase2_ns"] = r2.exec_time_ns

    out = np.empty((B, C, T, F), np.float32)
    for i in range(NCORES):
        b, q = divmod(i, 4)
        out[b, :, q * TQ : (q + 1) * TQ, :] = r2.results[i]["out"]
    return out



# revision 3
# speedup vs baseline: 19.8723x; 19.8723x over previous
"""Channel-attention Trainium2 kernel (Bass/Tile, 8 NeuronCores).

The reference computes, after un-permuting the V path:

    out[b,c,t,f] = sum_k w[b, f//64, c, k] * x[b,k,t,f]
    w[b,h]       = softmax_k( (q_h rows) @ (k_h rows)^T / 8 )
    q            = mean_t(x[b]) @ Wq.T + bq,   k = mean_t(x[b]) @ Wk.T

i.e. a per-(batch, head) 128x128 channel-mixing matmul over the full
(T x 64) feature block, fed by a tiny pooled q/k path.

Under axon the wall-clock is dominated by the host<->device tunnel
(~50-70 MB/s each way), so the design minimizes bytes crossed:

- The pooled q/k path (mean over T, two 512x512 projections, 8 softmaxes)
  is ~17 MFLOP -- computed on host in numpy; only the resulting
  (128, 8, 128) attention weights ship to each core.
- x ships once, as bf16 (128 MB instead of 512 MB for the fp32 two-phase
  scheme), and the output returns as bf16 (harness gate is 2e-2; bf16
  costs ~1e-3 here).
- One device program, one dispatch: each core takes a (b, T-quarter)
  shard xs[c, t, f] and streams 16 tiles of 8 t's, with one N=512
  matmul per head per tile into a rotating PSUM bank.

Sharding: 8 cores = (batch b in {0,1}) x (T-quarter tq in {0..3}).
"""

import numpy as np
import ml_dtypes

import concourse.bacc as bacc
import concourse.mybir as mybir
import concourse.tile as tile
from concourse.bass import ds, ts
from concourse.bass_utils import run_bass_kernel_spmd

B, C, T, F = 2, 128, 512, 512
H = 8
D = F // H            # 64 features per head
NCORES = 8
TQ = T // 4           # 128 t's per core
F32 = mybir.dt.float32
BF16 = mybir.dt.bfloat16
NPBF16 = ml_dtypes.bfloat16

TRACE = False
LAST_PROFILE = {}

_CACHE = {}


def _build(repeat=1):
    """Streaming channel-mix: out[c,t,f] = sum_k wt[k, f//64, c] xs[k,t,f].

    repeat>1 re-runs the streaming pass (same reads/writes), bench only.
    """
    nc = bacc.Bacc(
        "TRN2", target_bir_lowering=False, debug=False, num_devices=NCORES
    )
    xs = nc.dram_tensor("xs", [C, TQ, F], BF16, kind="ExternalInput")
    wt = nc.dram_tensor("wt", [C, H, C], BF16, kind="ExternalInput")  # w[b,h,c,k] at [k,h,c]
    out = nc.dram_tensor("out", [C, TQ, F], BF16, kind="ExternalOutput")
    TT = 8  # t's per tile: per head free = TT*D = 512 = one PSUM bank
    with tile.TileContext(nc) as tc:
        with (
            tc.tile_pool(name="wts", bufs=1) as wts,
            tc.tile_pool(name="xin", bufs=3) as xpool,
            tc.tile_pool(name="oout", bufs=3) as opool,
            tc.tile_pool(name="ps", bufs=8, space="PSUM") as psp,
        ):
            wt_sb = wts.tile([C, H, C], BF16, name="wt_sb")
            nc.sync.dma_start(wt_sb[:], wt[:])
            for rep in range(repeat):
                for it in range(TQ // TT):
                    xt = xpool.tile([C, TT, F], BF16, name="xt")
                    nc.sync.dma_start(xt[:], xs[:, ts(it, TT), :])
                    ot = opool.tile([C, TT, F], BF16, name="ot")
                    for h in range(H):
                        ps = psp.tile([C, TT, D], F32, name="ps")
                        nc.tensor.matmul(
                            ps[:],
                            wt_sb[:, h, :],
                            xt[:, :, ds(D * h, D)],
                            start=True,
                            stop=True,
                        )
                        if h % 2 == 0:
                            nc.vector.tensor_copy(ot[:, :, ds(D * h, D)], ps[:])
                        else:
                            nc.scalar.copy(ot[:, :, ds(D * h, D)], ps[:])
                    nc.scalar.dma_start(out[:, ts(it, TT), :], ot[:])
    nc.finalize()
    return nc


def _program():
    if "p" not in _CACHE:
        _CACHE["p"] = _build()
    return _CACHE["p"]


def _host_attention_weights(x, Wq, bq, Wk):
    """w[b,h] = softmax over k of the pooled q/k path; returns wt[b][k,h,c]."""
    xm = x.mean(axis=2)                      # (B,C,F) fp32
    q = xm @ Wq.T + bq                       # (B,C,F)
    k = xm @ Wk.T
    s = float(D) ** -0.25
    qh = q.reshape(B, C, H, D).transpose(0, 2, 1, 3) * s   # (B,H,C,D)
    kh = k.reshape(B, C, H, D).transpose(0, 2, 1, 3) * s
    logits = np.einsum("bhcd,bhkd->bhck", qh, kh, optimize=True)
    logits -= logits.max(axis=-1, keepdims=True)
    np.exp(logits, out=logits)
    logits /= logits.sum(axis=-1, keepdims=True)           # w (B,H,C,C)
    # device wants lhsT layout wt[k, h, c] = w[h, c, k]
    return [
        np.ascontiguousarray(logits[b].transpose(2, 0, 1)).astype(NPBF16)
        for b in range(B)
    ]


def kernel(x, Wq, bq, Wk):
    x = np.ascontiguousarray(np.asarray(x), dtype=np.float32)
    Wq = np.asarray(Wq, dtype=np.float32)
    bq = np.asarray(bq, dtype=np.float32)
    Wk = np.asarray(Wk, dtype=np.float32)
    assert x.shape == (B, C, T, F)

    nc = _program()
    wt_list = _host_attention_weights(x, Wq, bq, Wk)

    xbf = x.astype(NPBF16)
    in_maps = []
    for i in range(NCORES):
        b, tq = divmod(i, 4)
        in_maps.append(
            {
                "xs": np.ascontiguousarray(xbf[b, :, tq * TQ : (tq + 1) * TQ, :]),
                "wt": wt_list[b],
            }
        )

    r = run_bass_kernel_spmd(nc, in_maps, list(range(NCORES)), trace=TRACE)
    LAST_PROFILE["exec_ns"] = r.exec_time_ns

    out = np.empty((B, C, T, F), np.float32)
    for i in range(NCORES):
        b, tq = divmod(i, 4)
        out[b, :, tq * TQ : (tq + 1) * TQ, :] = r.results[i]["out"]
    return out


# revision 9
# speedup vs baseline: 28.5007x; 1.4342x over previous
"""Channel-attention Trainium2 kernel (Bass/Tile, 8 NeuronCores).

The reference computes, after un-permuting the V path:

    out[b,c,t,f] = sum_k w[b, f//64, c, k] * x[b,k,t,f]
    w[b,h]       = softmax_k( (q_h rows) @ (k_h rows)^T / 8 )
    q            = mean_t(x[b]) @ Wq.T + bq,   k = mean_t(x[b]) @ Wk.T

i.e. a per-(batch, head) 128x128 channel-mixing matmul over the full
(T x 64) feature block, fed by a tiny pooled q/k path.

Under axon the wall-clock is dominated by the host<->device tunnel
(~50-70 MB/s each way), so the design minimizes bytes crossed:

- The pooled q/k path (~17 MFLOP) runs on host; only the (128, 8, 128)
  attention weights ship per core.
- x ships as int8 with a per-(t,f)-column scale folded on the host
  (the scale cancels through the channel mix, so the device never sees
  it). 64 MB instead of 256 MB fp32.
- The device computes M = w @ x_q with x as the *stationary* matmul
  operand, so M lands in PSUM transposed: columns on partitions. That
  makes the per-column absmax (reduce with apply_absolute_value) and the
  127/max quantization (tensor_scalar per-partition multiply) native
  vector ops. The output returns as int8 plus a (TQ, F) fp32 scale
  plane; the store DMA de-transposes back to natural [C, TQ, F] layout
  (two affine APs, split on the t-within-pair bit of the partition
  index).
- Host dequant: out = q * scale/127, threaded across shards.

Measured end-to-end rel err vs the fp32 reference: ~1e-2 (gate 2e-2).
Sharding: 8 cores = (batch b in {0,1}) x (T-quarter tq in {0..3}).
"""

from concurrent.futures import ThreadPoolExecutor

import numpy as np
import ml_dtypes

import concourse.bacc as bacc
import concourse.mybir as mybir
import concourse.tile as tile
from concourse.bass import ds, ts
from concourse.bass_utils import run_bass_kernel_spmd
from concourse.masks import make_identity

B, C, T, F = 2, 128, 512, 512
H = 8
D = F // H            # 64 features per head
NCORES = 8
TQ = T // 4           # 128 t's per core
TT = 8                # t's per device tile
NTILES = TQ // TT     # 16
F32 = mybir.dt.float32
BF16 = mybir.dt.bfloat16
I8 = mybir.dt.int8
NPBF16 = ml_dtypes.bfloat16

TRACE = False
LAST_PROFILE = {}

_CACHE = {}


def _build(repeat=1):
    """Transposed channel-mix with int8 I/O and per-column scales.

    M[col, c] = sum_k xs[k, col] w[c, k]  (col = (t, d) within head h)
    rm[col]   = max_c |M|
    q[col, c] = int8(round(M * 127 / rm))
    """
    nc = bacc.Bacc(
        "TRN2", target_bir_lowering=False, debug=False, num_devices=NCORES
    )
    xs = nc.dram_tensor("xs", [C, TQ, F], I8, kind="ExternalInput")
    wt = nc.dram_tensor("wt", [C, H, C], BF16, kind="ExternalInput")  # w[b,h,c,k] at [k,h,c]
    oq = nc.dram_tensor("oq", [C, TQ, F], I8, kind="ExternalOutput")
    # device-native scale layout; host reindexes (t = it*TT + 2j + p//64,
    # f = h*64 + p%64)
    sc = nc.dram_tensor("sc", [128, NTILES, H, TT // 2], F32, kind="ExternalOutput")
    with tile.TileContext(nc) as tc:
        with (
            tc.tile_pool(name="wts", bufs=1) as wts,
            tc.tile_pool(name="xin", bufs=3) as xpool,
            tc.tile_pool(name="xbf", bufs=2) as xbpool,
            tc.tile_pool(name="qb", bufs=3) as qbpool,
            tc.tile_pool(name="oout", bufs=3) as opool,
            tc.tile_pool(name="sout", bufs=3) as spool,
            tc.tile_pool(name="rq", bufs=2) as rqpool,
            tc.tile_pool(name="psA", bufs=4, space="PSUM") as pspA,
            tc.tile_pool(name="psB", bufs=4, space="PSUM") as pspB,
        ):
            ident = wts.tile([128, 128], F32, name="ident")
            make_identity(nc, ident)
            wt_sb = wts.tile([C, H, C], BF16, name="wt_sb")
            nc.sync.dma_start(wt_sb[:], wt[:])
            for rep in range(repeat):
                for it in range(NTILES):
                    xt = xpool.tile([C, TT, F], I8, name="xt")
                    nc.sync.dma_start(xt[:], xs[:, ts(it, TT), :])
                    # conversion copies also regroup each head's (t-pair, d)
                    # columns contiguously: the matmul stationary operand
                    # needs a single free dimension
                    xb = xbpool.tile([C, H, TT // 2, 2, D], BF16, name="xb")
                    for h in range(H):
                        nc.scalar.copy(
                            xb[:, h],
                            xt[:, :, ds(D * h, D)].rearrange(
                                "k (j u) v -> k j u v", u=2
                            ),
                        )
                    ot = opool.tile([C, TT, F], I8, name="ot")
                    st = spool.tile([128, H, TT // 2], F32, name="st")
                    for h in range(H):
                        pt = pspA.tile([128, TT // 2, C], F32, name="pt")
                        for j in range(TT // 2):
                            # stationary: x columns (2 t's, 64 d's) of head h
                            nc.tensor.matmul(
                                pt[:, j, :],
                                xb[:, h, j],
                                wt_sb[:, h, :],
                                start=True,
                                stop=True,
                            )
                        nc.vector.reduce_max(
                            st[:, h, :],
                            pt[:],
                            axis=mybir.AxisListType.X,
                            apply_absolute_value=True,
                        )
                        rq = rqpool.tile([128, TT // 2], F32, name="rq")
                        nc.vector.reciprocal(rq[:], st[:, h, :])
                        qb = qbpool.tile([128, TT // 2, C], F32, name="qb")
                        for j in range(TT // 2):
                            nc.vector.tensor_scalar(
                                qb[:, j, :],
                                pt[:, j, :],
                                rq[:, j : j + 1],
                                127.0,
                                op0=mybir.AluOpType.mult,
                                op1=mybir.AluOpType.mult,
                            )
                        for j in range(TT // 2):
                            ptt = pspB.tile([128, 128], F32, name="ptt")
                            nc.tensor.transpose(ptt[:], qb[:, j, :], ident[:])
                            dst = ot[:, ts(j, 2), ds(D * h, D)]
                            src = ptt[:].rearrange("c (u v) -> c u v", u=2)
                            if h % 2 == 0:
                                nc.scalar.copy(dst, src)
                            else:
                                nc.vector.tensor_copy(dst, src)
                    nc.scalar.dma_start(oq[:, ts(it, TT), :], ot[:])
                    nc.sync.dma_start(sc[:, it, :, :], st[:])
    nc.finalize()
    return nc


def _program():
    if "p" not in _CACHE:
        _CACHE["p"] = _build()
    return _CACHE["p"]


def _host_attention_weights(x, Wq, bq, Wk):
    """w[b,h] = softmax over k of the pooled q/k path; returns wt[b][k,h,c]."""
    xm = x.mean(axis=2)                      # (B,C,F) fp32
    q = xm @ Wq.T + bq                       # (B,C,F)
    k = xm @ Wk.T
    s = float(D) ** -0.25
    qh = q.reshape(B, C, H, D).transpose(0, 2, 1, 3) * s   # (B,H,C,D)
    kh = k.reshape(B, C, H, D).transpose(0, 2, 1, 3) * s
    logits = np.einsum("bhcd,bhkd->bhck", qh, kh, optimize=True)
    logits -= logits.max(axis=-1, keepdims=True)
    np.exp(logits, out=logits)
    logits /= logits.sum(axis=-1, keepdims=True)           # w (B,H,C,C)
    # device wants lhsT layout wt[k, h, c] = w[h, c, k]
    return [
        np.ascontiguousarray(logits[b].transpose(2, 0, 1)).astype(NPBF16)
        for b in range(B)
    ]


def _quantize_shard(args):
    x, b, tq = args
    xsl = x[b, :, tq * TQ : (tq + 1) * TQ, :]
    cin = np.abs(xsl).max(axis=0)                    # (TQ, F)
    np.maximum(cin, 1e-30, out=cin)
    q = np.rint(xsl * (127.0 / cin)).astype(np.int8)
    return q, cin


def _dequant_shard(args):
    out, b, tq, q, scale = args
    out[b, :, tq * TQ : (tq + 1) * TQ, :] = q.astype(np.float32) * (
        scale * (1.0 / 127.0)
    )


def kernel(x, Wq, bq, Wk):
    x = np.ascontiguousarray(np.asarray(x), dtype=np.float32)
    Wq = np.asarray(Wq, dtype=np.float32)
    bq = np.asarray(bq, dtype=np.float32)
    Wk = np.asarray(Wk, dtype=np.float32)
    assert x.shape == (B, C, T, F)

    nc = _program()
    wt_list = _host_attention_weights(x, Wq, bq, Wk)

    shard_bt = [divmod(i, 4) for i in range(NCORES)]
    with ThreadPoolExecutor(NCORES) as ex:
        quants = list(ex.map(_quantize_shard, [(x, b, tq) for b, tq in shard_bt]))

    in_maps = [
        {"xs": quants[i][0], "wt": wt_list[shard_bt[i][0]]} for i in range(NCORES)
    ]

    r = run_bass_kernel_spmd(nc, in_maps, list(range(NCORES)), trace=TRACE)
    LAST_PROFILE["exec_ns"] = r.exec_time_ns

    out = np.empty((B, C, T, F), np.float32)
    jobs = []
    for i in range(NCORES):
        b, tq = shard_bt[i]
        # sc comes back in device-native [p, it, h, j] layout:
        # t = it*TT + 2j + p//64, f = h*64 + p%64
        grid = (
            r.results[i]["sc"]
            .reshape(2, 64, NTILES, H, TT // 2)
            .transpose(2, 4, 0, 3, 1)
            .reshape(TQ, F)
        )
        # device scale (col absmax of M) already includes the host input
        # scale factored out: out = q_dev * sc_dev/127 * cin/127
        full_scale = grid * (quants[i][1] * (1.0 / 127.0))
        jobs.append((out, b, tq, r.results[i]["oq"], full_scale))
    with ThreadPoolExecutor(NCORES) as ex:
        list(ex.map(_dequant_shard, jobs))
    return out
